# revision 2
# baseline (speedup 1.0000x reference)
"""Trainium2 Bass kernel for nn_BilinearLabelAttention.

out[b,l,i,o] = sum_j head[b,i,j] * label_U_diag[l,j] * dep[b,o,j]
  head/dep: [8, 512, 512] f32, label_U_diag: [32, 512] f32
  out: [8, 32, 512, 512] f32

Sharding: data-parallel over batch — core b computes out[b].

Compute layout (dep-stationary, transposed output): per label l,
  psumT[l][o,i] = sum_kt dep[kt][:,o]^T @ (head[kt] * u_l[kt])
so the PE stationary operand is a dep column block dep[kt][:, oi*128:+128]
shared by consecutive matmuls (labels are processed in pairs with kt as the
outer loop), giving every LDWEIGHTS a 2-matmul (~426ns) window to hide in —
the i-stationary form reloads weights every matmul and slips ~21ns each.
The output is written transposed ([L, o, i]) and un-transposed on the host.

Everything off the PE accumulator runs in bf16: inputs are converted on the
host (halves input DMA, and bf16 operands need no on-device f32r rounding
copy), matmuls take bf16 at the same 1 cycle/row as f32r, and outputs are
evacuated from PSUM as bf16 and upcast on the host — halving the 33.6MB/core
output DMA that sits right at the roofline ridge. Max rel err ~4e-3 vs the
2e-2 gate.

Engine budget per core: PE 512 matmuls ~109us (bottleneck), DVE 128
label-scalings (bf16 2x mode) + 1/4 of evacs, ACT 3/4 of evacs, out-DMA
triggers alternate sync/gpsimd queues. Scalings are emitted two label-pairs
ahead so DVE never gates the PE at group boundaries.
"""

import os

import ml_dtypes
import numpy as np

os.environ.setdefault("BASS_NEVER_TRACE", "1")

import concourse.bass as bass
import concourse.mybir as mybir
from concourse.bass_utils import run_bass_kernel_spmd
from concourse.tile import TileContext
from concourse.vector_clock import ScopedClock

B, S, D, L = 8, 512, 512, 32
P = 128
KT = D // P  # 4 contraction tiles
OI = S // P  # 4 output-column tiles
NPAIR = L // 2  # 16 label pairs

_BF16 = ml_dtypes.bfloat16


class _LeanTailTileContext(TileContext):
    """TileContext exit without the second all-engine barrier: engines with
    nothing left simply halt; semaphore clears still happen after the
    pre-clear barrier, so repeat executions stay correct."""

    def _drain_and_barrier(self, tick_clock, wait_clock):
        drain_inst = self.nc.sync.drain()
        wait_clock.add_sem_waits(
            drain_inst.ins, ScopedClock({None: tick_clock.global_clock})
        )
        self.nc.all_engine_barrier()
        assert self.sems is not None
        popped = self.nc._tile_sem_poison_stack.pop()
        assert popped is self._sem_poison
        self.nc.clear_and_free_semaphores(list(self.sems.allocated().values()))


def _spread_multi_waits(nc):
    """The walrus build in this container accepts at most ONE semaphore wait
    per instruction ("Too many sync wait commands"). Hoist all-but-one wait
    of each multi-wait instruction onto single-wait NoOps inserted before it
    on the same engine queue (engines execute in order, so gating the queue
    earlier is equivalent)."""
    for f in nc.m.functions:
        for bb in f.blocks:
            new_insts = []
            for ins in bb.instructions:
                w = list(ins.sync_info.on_wait) if ins.sync_info else []
                if len(w) > 1:
                    for extra in w[:-1]:
                        nop = mybir.InstNoOp(
                            name=nc.get_next_instruction_name(), ins=[], outs=[]
                        )
                        nop.engine = ins.engine
                        nop.sync_info = mybir.SyncInfo(on_wait=[extra], on_update=[])
                        new_insts.append(nop)
                    ins.sync_info.on_wait = [w[-1]]
                new_insts.append(ins)
            bb.instructions[:] = new_insts


def _strip_const_memsets(nc):
    """Bass's preamble memsets four const-* SBUF tiles this kernel never
    reads; they run through the GpSimd DGE queue and hold the entry barrier
    behind ~3.5us of cold-queue latency. Drop them."""
    bb = nc.m.functions[0].blocks[0]
    bb.instructions[:] = [
        ins
        for ins in bb.instructions
        if not (
            type(ins).__name__ == "InstMemset"
            and str(ins.engine).endswith("Pool")
            and not ins.sync_info
        )
    ]


def _build():
    f32 = mybir.dt.float32
    bf16 = mybir.dt.bfloat16

    nc = bass.Bass(enable_partition_id=False)
    headT = nc.declare_dram_parameter("headT", [D, S], bf16, isOutput=False)
    depT = nc.declare_dram_parameter("depT", [D, S], bf16, isOutput=False)
    uT = nc.declare_dram_parameter("uT", [D, L], f32, isOutput=False)
    # Transposed output: outT[l, o, i]; host swaps the last two axes back.
    outT = nc.declare_dram_parameter("outT", [L, S, S], bf16, isOutput=True)

    with _LeanTailTileContext(nc) as tc:
        with (
            tc.tile_pool(name="inputs", bufs=1) as in_pool,
            tc.tile_pool(name="scaled", bufs=24) as sc_pool,
            tc.tile_pool(name="outs", bufs=16) as out_pool,
            tc.tile_pool(name="psum", bufs=8, space="PSUM") as ps_pool,
        ):
            # Input loads: kt0 and kt1 as separate DMAs (they gate the first
            # matmuls), kt2-3 batched into one strided DMA. dep on sync/HWDGE,
            # head on scalar, u on gpsimd — the three queues issue in parallel.
            def load_tensor(dram, eng, tagp):
                t0_ = in_pool.tile([P, S], bf16, name=f"{tagp}0", tag=f"{tagp}0")
                eng.dma_start(out=t0_[:], in_=dram[0:P, :])
                t1_ = in_pool.tile([P, S], bf16, name=f"{tagp}1", tag=f"{tagp}1")
                eng.dma_start(out=t1_[:], in_=dram[P : 2 * P, :])
                t23 = in_pool.tile([P, 2 * S], bf16, name=f"{tagp}23", tag=f"{tagp}23")
                eng.dma_start(
                    out=t23[:].rearrange("p (kt o) -> p kt o", kt=2),
                    in_=dram[2 * P : 4 * P, :].rearrange("(kt p) o -> p kt o", p=P),
                )
                return [t0_[:], t1_[:], t23[:, :S], t23[:, S:]]

            dep_sb = load_tensor(depT, nc.sync, "dep")
            u_all = in_pool.tile([P, KT * L], f32, name="u_all", tag="u_all")
            nc.gpsimd.dma_start(
                out=u_all[:].rearrange("p (kt l) -> p kt l", kt=KT),
                in_=uT.rearrange("(kt p) l -> p kt l", p=P),
            )
            u_sb = [u_all[:, kt * L : (kt + 1) * L] for kt in range(KT)]
            head_sb = load_tensor(headT, nc.scalar, "head")

            scaled = {}

            def emit_scalings(pair):
                # Production order matches PE consumption (kt outer), so the
                # first matmuls of a pair are never left waiting on DVE.
                for kt in range(KT):
                    for l in (2 * pair, 2 * pair + 1):
                        s = sc_pool.tile(
                            [P, S], bf16, name=f"s_{l}_{kt}", tag="s"
                        )
                        nc.vector.tensor_scalar_mul(
                            s[:], head_sb[kt][:], u_sb[kt][:, l : l + 1]
                        )
                        scaled[(l, kt)] = s

            evac_idx = 0

            def evac(l, oi, ps, last_pair):
                nonlocal evac_idx
                ot = out_pool.tile([P, S], bf16, name=f"ot_{l}_{oi}", tag="ot")
                # ~3:1 ACT:DVE split keeps both far under the PE stream;
                # the final pair alternates so the tail drain runs in
                # parallel on both engines.
                use_dve = (l % 2 == 1) if last_pair else (evac_idx % 4 == 3)
                if use_dve:
                    nc.vector.tensor_copy(out=ot[:], in_=ps[:])
                else:
                    nc.scalar.copy(ot[:], ps[:])
                deng = nc.sync if evac_idx % 2 == 0 else nc.gpsimd
                deng.dma_start(out=outT[l, oi * P : (oi + 1) * P, :], in_=ot[:])
                evac_idx += 1

            emit_scalings(0)
            emit_scalings(1)
            for pair in range(NPAIR):
                l0 = 2 * pair
                psums = [
                    ps_pool.tile([P, S], f32, name=f"ps_{l0}_{x}", tag="ps")
                    for x in range(2 * OI)
                ]  # index oi*2 + li
                for kt in range(KT):
                    for oi in range(OI):
                        w = dep_sb[kt][:, oi * P : (oi + 1) * P]
                        for li in range(2):
                            ps = psums[oi * 2 + li]
                            nc.tensor.matmul(
                                ps[:],
                                lhsT=w,
                                rhs=scaled[(l0 + li, kt)][:],
                                start=(kt == 0),
                                stop=(kt == KT - 1),
                            )
                            if kt == KT - 1:
                                evac(l0 + li, oi, ps, pair == NPAIR - 1)
                for kt in range(KT):
                    for li in range(2):
                        del scaled[(l0 + li, kt)]
                if pair + 2 < NPAIR:
                    emit_scalings(pair + 2)

    _strip_const_memsets(nc)
    _spread_multi_waits(nc)
    return nc


def _prepare_in_maps(head, dep, label_U_diag):
    head = np.asarray(head, dtype=np.float32)
    dep = np.asarray(dep, dtype=np.float32)
    u = np.asarray(label_U_diag, dtype=np.float32)
    uT = np.ascontiguousarray(u.T)  # [D, L] f32
    return [
        {
            "headT": np.ascontiguousarray(head[b].T).astype(_BF16),
            "depT": np.ascontiguousarray(dep[b].T).astype(_BF16),
            "uT": uT,
        }
        for b in range(B)
    ]


def _postprocess(results):
    # outT[l, o, i] bf16 -> out[l, i, o] f32
    return np.stack(
        [
            np.asarray(results[b]["outT"]).astype(np.float32).transpose(0, 2, 1)
            for b in range(B)
        ]
    )


_NC_CACHE = None


def kernel(head, dep, label_U_diag):
    global _NC_CACHE
    in_maps = _prepare_in_maps(head, dep, label_U_diag)
    if _NC_CACHE is None:
        _NC_CACHE = _build()
    res = run_bass_kernel_spmd(_NC_CACHE, in_maps, list(range(B)), trace=False)
    return _postprocess(res.results)


# revision 3
# speedup vs baseline: 1.2873x; 1.2873x over previous
"""Trainium2 Bass kernel for nn_BilinearLabelAttention.

out[b,l,i,o] = sum_j head[b,i,j] * label_U_diag[l,j] * dep[b,o,j]
  head/dep: [8, 512, 512] f32, label_U_diag: [32, 512] f32
  out: [8, 32, 512, 512] f32

Sharding: data-parallel over batch — core b computes out[b]. Per core that
is L=32 matmuls of (head*diag(U_l)) @ dep^T, i.e. 512 PE matmuls of
[128j,128i]^T @ [128j,512o] accumulated over 4 j-tiles in PSUM, with the
4-matmul accumulation chain kept consecutive per PSUM bank (interleaving
accumulation groups / switching banks every matmul costs ~46ns per matmul
in PE pipeline bubbles — measured).

Everything off the PE accumulator runs in bf16: inputs are converted on the
host (halves input DMA; bf16 operands need no on-device f32r rounding copy
and LDWEIGHTS at 117ns hides fully under the 213ns matmul window, where the
f32r 187ns load slipped ~10ns/matmul), and outputs are evacuated from PSUM
as bf16 and upcast on the host — halving the 33.6MB/core output DMA that
sits right at the roofline ridge. Max rel err ~4e-3 vs the 2e-2 gate.

label_U_diag is pre-shuffled on the host into the exact [128, KT*L] SBUF
layout so its DMA is one contiguous 512B descriptor per partition (the
on-device rearrange was 128B-element gather that landed ~3us late and
stalled the whole DVE scaling chain behind it).

Out-tiles are written in [128, 1024] pairs (two 128-row chunks per DMA) and
all out-DMA triggers stay on the sync queue: gpsimd's software DGE takes
~7.7us to DRAIN at kernel exit if it holds DMA queue entries. Evacuation
splits ~3:1 ACT:DVE; the last label alternates engines so the tail drain
runs in parallel.
"""

import os

import ml_dtypes
import numpy as np

os.environ.setdefault("BASS_NEVER_TRACE", "1")

import concourse.bass as bass
import concourse.mybir as mybir
from concourse.bass_utils import run_bass_kernel_spmd
from concourse.tile import TileContext
from concourse.vector_clock import ScopedClock

B, S, D, L = 8, 512, 512, 32
P = 128
KT = D // P
MT = S // P

_BF16 = ml_dtypes.bfloat16


class _LeanTailTileContext(TileContext):
    """TileContext exit without the second all-engine barrier: engines with
    nothing left simply halt; semaphore clears still happen after the
    pre-clear barrier, so repeat executions stay correct."""

    def _drain_and_barrier(self, tick_clock, wait_clock):
        drain_inst = self.nc.sync.drain()
        wait_clock.add_sem_waits(
            drain_inst.ins, ScopedClock({None: tick_clock.global_clock})
        )
        self.nc.all_engine_barrier()
        assert self.sems is not None
        popped = self.nc._tile_sem_poison_stack.pop()
        assert popped is self._sem_poison
        self.nc.clear_and_free_semaphores(list(self.sems.allocated().values()))


def _spread_multi_waits(nc):
    """The walrus build in this container accepts at most ONE semaphore wait
    per instruction ("Too many sync wait commands"). Hoist all-but-one wait
    of each multi-wait instruction onto single-wait NoOps inserted before it
    on the same engine queue (engines execute in order, so gating the queue
    earlier is equivalent)."""
    for f in nc.m.functions:
        for bb in f.blocks:
            new_insts = []
            for ins in bb.instructions:
                w = list(ins.sync_info.on_wait) if ins.sync_info else []
                if len(w) > 1:
                    for extra in w[:-1]:
                        nop = mybir.InstNoOp(
                            name=nc.get_next_instruction_name(), ins=[], outs=[]
                        )
                        nop.engine = ins.engine
                        nop.sync_info = mybir.SyncInfo(on_wait=[extra], on_update=[])
                        new_insts.append(nop)
                    ins.sync_info.on_wait = [w[-1]]
                new_insts.append(ins)
            bb.instructions[:] = new_insts


def _strip_const_memsets(nc):
    """Bass's preamble memsets four const-* SBUF tiles this kernel never
    reads; they run through the GpSimd DGE queue and hold the entry barrier
    behind ~3.5us of cold-queue latency. Drop them."""
    bb = nc.m.functions[0].blocks[0]
    bb.instructions[:] = [
        ins
        for ins in bb.instructions
        if not (
            type(ins).__name__ == "InstMemset"
            and str(ins.engine).endswith("Pool")
            and not ins.sync_info
        )
    ]


def _build():
    f32 = mybir.dt.float32
    bf16 = mybir.dt.bfloat16

    nc = bass.Bass(enable_partition_id=False)
    headT = nc.declare_dram_parameter("headT", [D, S], bf16, isOutput=False)
    depT = nc.declare_dram_parameter("depT", [D, S], bf16, isOutput=False)
    uH = nc.declare_dram_parameter("uH", [P, KT * L], f32, isOutput=False)
    out = nc.declare_dram_parameter("out", [L, S, S], bf16, isOutput=True)

    with _LeanTailTileContext(nc) as tc:
        with (
            tc.tile_pool(name="inputs", bufs=1) as in_pool,
            tc.tile_pool(name="scaled", bufs=4) as sc_pool,
            tc.tile_pool(name="outs", bufs=8) as out_pool,
            tc.tile_pool(name="psum", bufs=8, space="PSUM") as ps_pool,
        ):
            # Input loads: kt0 and kt1 as separate small DMAs (they gate the
            # first matmuls), kt2-3 batched into one strided DMA. u first on
            # sync (tiny, gates every DVE scaling), dep on sync, head on
            # scalar — the two queues issue in parallel.
            u_all = in_pool.tile([P, KT * L], f32, name="u_all", tag="u_all")
            nc.sync.dma_start(out=u_all[:], in_=uH[:, :])
            u_sb = [u_all[:, kt * L : (kt + 1) * L] for kt in range(KT)]

            def load_tensor(dram, eng, tagp):
                t0_ = in_pool.tile([P, S], bf16, name=f"{tagp}0", tag=f"{tagp}0")
                eng.dma_start(out=t0_[:], in_=dram[0:P, :])
                t1_ = in_pool.tile([P, S], bf16, name=f"{tagp}1", tag=f"{tagp}1")
                eng.dma_start(out=t1_[:], in_=dram[P : 2 * P, :])
                t23 = in_pool.tile([P, 2 * S], bf16, name=f"{tagp}23", tag=f"{tagp}23")
                eng.dma_start(
                    out=t23[:].rearrange("p (kt o) -> p kt o", kt=2),
                    in_=dram[2 * P : 4 * P, :].rearrange("(kt p) o -> p kt o", p=P),
                )
                return [t0_[:], t1_[:], t23[:, :S], t23[:, S:]]

            dep_sb = load_tensor(depT, nc.sync, "dep")
            head_sb = load_tensor(headT, nc.scalar, "head")

            def make_scaled(l, kt):
                s = sc_pool.tile([P, S], bf16, name=f"s_{l}_{kt}", tag=f"scaled{kt}")
                if l == 0:
                    # Quarter granularity on the first label so the first
                    # matmul waits only on a quarter of head[kt].
                    for mi in range(MT):
                        sl = slice(mi * P, (mi + 1) * P)
                        nc.vector.tensor_scalar_mul(
                            s[:, sl], head_sb[kt][:, sl], u_sb[kt][:, l : l + 1]
                        )
                else:
                    nc.vector.tensor_scalar_mul(
                        s[:], head_sb[kt][:], u_sb[kt][:, l : l + 1]
                    )
                return s

            otile = [None]

            def evac(l, mi, ps):
                # Pair two 128-row chunks into one [P, 2S] tile so each DMA
                # moves 2KB/partition with a single sync-queue trigger.
                if mi % 2 == 0:
                    otile[0] = out_pool.tile([P, 2 * S], bf16, name=f"ot_{l}_{mi}", tag="ot")
                ot = otile[0]
                sl = slice((mi % 2) * S, (mi % 2 + 1) * S)
                use_dve = (mi % 2 == 1) if l == L - 1 else (mi == 3)
                if use_dve:
                    nc.vector.tensor_copy(out=ot[:, sl], in_=ps[:])
                else:
                    nc.scalar.copy(ot[:, sl], ps[:])
                if mi % 2 == 1:
                    half = mi // 2
                    nc.sync.dma_start(
                        out=out[l, half * 2 * P : (half + 1) * 2 * P, :].rearrange(
                            "(two p) o -> p two o", p=P
                        ),
                        in_=ot[:].rearrange("p (two o) -> p two o", two=2),
                    )

            for l in range(L):
                scaled = [make_scaled(l, kt) for kt in range(KT)]
                if l == 0:
                    # kt-outer for the first label: its first matmuls need
                    # only the kt=0 input tiles (which land first).
                    psums = [
                        ps_pool.tile([P, S], f32, name=f"ps_{l}_{mi}", tag="ps")
                        for mi in range(MT)
                    ]
                    for kt in range(KT):
                        for mi in range(MT):
                            nc.tensor.matmul(
                                psums[mi][:],
                                lhsT=scaled[kt][:, mi * P : (mi + 1) * P],
                                rhs=dep_sb[kt][:],
                                start=(kt == 0),
                                stop=(kt == KT - 1),
                            )
                    for mi in range(MT):
                        evac(l, mi, psums[mi])
                    continue
                for mi in range(MT):
                    ps = ps_pool.tile([P, S], f32, name=f"ps_{l}_{mi}", tag="ps")
                    for kt in range(KT):
                        nc.tensor.matmul(
                            ps[:],
                            lhsT=scaled[kt][:, mi * P : (mi + 1) * P],
                            rhs=dep_sb[kt][:],
                            start=(kt == 0),
                            stop=(kt == KT - 1),
                        )
                    evac(l, mi, ps)

    _strip_const_memsets(nc)
    _spread_multi_waits(nc)
    return nc


def _prepare_in_maps(head, dep, label_U_diag):
    head = np.asarray(head, dtype=np.float32)
    dep = np.asarray(dep, dtype=np.float32)
    u = np.asarray(label_U_diag, dtype=np.float32)
    # uH[p, kt*L + l] = u[l, kt*P + p] — the exact SBUF tile layout.
    uH = np.ascontiguousarray(
        u.T.reshape(KT, P, L).transpose(1, 0, 2).reshape(P, KT * L)
    )
    return [
        {
            "headT": np.ascontiguousarray(head[b].T).astype(_BF16),
            "depT": np.ascontiguousarray(dep[b].T).astype(_BF16),
            "uH": uH,
        }
        for b in range(B)
    ]


def _postprocess(results):
    return np.stack(
        [np.asarray(results[b]["out"]).astype(np.float32) for b in range(B)]
    )


_NC_CACHE = None


def kernel(head, dep, label_U_diag):
    global _NC_CACHE
    in_maps = _prepare_in_maps(head, dep, label_U_diag)
    if _NC_CACHE is None:
        _NC_CACHE = _build()
    res = run_bass_kernel_spmd(_NC_CACHE, in_maps, list(range(B)), trace=False)
    return _postprocess(res.results)
